# revision 41
# baseline (speedup 1.0000x reference)
"""GAT-style attention head via bucketed suffix-sum tables, 8 TRN2 cores.

Math (per batch b):
    S   = seq @ Wf                     [N, D]
    f1  = S @ w1 + b1,  f2 = S @ w2 + b2        [N]
    t   = f1[:, None] + f2[None, :]    [N, N]
    e   = max(exp(t), exp(0.2 t))
        = exp(t)      where f2_j >= -f1_i   (branch A)
        = exp(0.2 t)  otherwise             (branch C)
    out = lrelu((e @ S) / rowsum(e) + bias)

Both branches are rank-1:  exp(t) = a_i b_j,  exp(.2t) = c_i d_j  with
a=exp(f1+b1), b=exp(f2+b2), c=a^.2, d=b^.2.  The A/C split is a threshold
on f2_j vs theta_i = -(f1_i+b1+b2).  Quantize thresholds onto a fixed grid
of G buckets; then

    e @ [S|1] (row i) ~= a_i * P(g_i) + c_i * (FullD - Q(g_i))

where P(g) = sum_{f2_j >= grid_g} b_j [S|1]_j and Q(g) likewise with d_j —
both are suffix sums of per-bucket tables, additive over j, so each core
builds tables over its own rows and a small AllReduce(+) combines them.
Misclassified pairs have |t| < bucket width; measured rel err ~3e-3.

O(N^2 D) dense work and the [B,N,D] AllGather are gone entirely.

Schedule notes:
- Tables are combined with an AllGather (3 doubling rounds, lower latency
  than the 6-round RDH AllReduce) plus a cheap local vector sum.
- The tau/broadcast/Sr work needed only by the post-collective gather is
  issued AFTER the collective so it overlaps the transfer window.
"""

import os
import sys
import numpy as np

if "/opt/trn_rl_repo" not in sys.path:
    sys.path.insert(0, "/opt/trn_rl_repo")

B, N, F, D = 2, 8192, 256, 128
CORES = 8
NL = N // CORES          # 1024 rows per core per batch
IT = NL // 128           # 8 row-tiles per core per batch
ALPHA = 0.2
G = 64                   # threshold grid buckets
GE = G + 1               # grid edges
LO, HI = -8.0, 8.0       # covers f1/f2 range (+-4.3 actual) with ~2x margin
GH = (HI - LO) / G
WB = 2 * (D + 1)         # 258: [S*b | b | S*d | d] table width
CCB = G * WB             # per-batch AllReduce payload (bf16 elems)

_cache = {}


def build(skip_collective=False):
    import concourse.bass as bass
    import concourse.bacc as bacc
    import concourse.mybir as mybir
    import concourse.tile as tile
    from concourse.masks import make_identity

    f32 = mybir.dt.float32
    bf16 = mybir.dt.bfloat16
    i32 = mybir.dt.int32
    AF = mybir.ActivationFunctionType
    ALU = mybir.AluOpType

    nc = bacc.Bacc(None, debug=False, num_devices=CORES)

    seq_ext = nc.declare_dram_parameter("seq", [B, NL, F], f32, isOutput=False)
    wf_ext = nc.declare_dram_parameter("Wf", [F, D], f32, isOutput=False)
    w1_ext = nc.declare_dram_parameter("w1", [D, 1], f32, isOutput=False)
    b1_ext = nc.declare_dram_parameter("b1", [1], f32, isOutput=False)
    w2_ext = nc.declare_dram_parameter("w2", [D, 1], f32, isOutput=False)
    b2_ext = nc.declare_dram_parameter("b2", [1], f32, isOutput=False)
    bias_ext = nc.declare_dram_parameter("bias", [D], f32, isOutput=False)
    out_ext = nc.declare_dram_parameter("out", [B, NL, D], f32, isOutput=True)

    with tile.TileContext(nc) as tc:
        persist_pool = tc.tile_pool(name="persist", bufs=1)
        pers = persist_pool.__enter__()

        def T(shape, dtype, name):
            return pers.tile(shape, dtype, tag=name, name=name)

        with tc.tile_pool(name="dram", bufs=1, space="DRAM") as dram:
            shared = "Local" if skip_collective else "Shared"
            cc_in = dram.tile([B * CCB], bf16, name="cc_in")
            cc_out = dram.tile([CORES * B * CCB], bf16, addr_space=shared,
                               name="cc_out")

            # ---------- persistent SBUF ----------
            ident = T([128, 128], bf16, name="ident")
            ones_row = T([1, 128], bf16, name="ones_row")    # outer-product lhsT
            ones_row_f = T([1, 128], f32, name="ones_row_f")
            wf_sb = T([128, F], f32, name="wf_sb")
            wf_bf = T([128, F], bf16, name="wf_bf")
            w12_st = T([128, 2], f32, name="w12_st")
            w12_bf = T([128, 2], bf16, name="w12_bf")        # [-w1 | w2]
            scal = T([128, 8], f32, name="scal")
            b1_sb = scal[0:1, 0:1]
            b2_sb = scal[0:1, 1:2]
            b12 = scal[0:1, 2:3]
            b1_bc = scal[:, 3:4]
            b2_bc = scal[:, 4:5]
            p2b1_bc = scal[:, 5:6]
            p2b2_bc = scal[:, 6:7]
            b12_bc = scal[:, 7:8]
            alpha_col = T([128, 1], f32, name="alpha_col")
            giota = T([128, 1], i32, name="giota")
            giota_f = T([128, 1], f32, name="giota_f")
            grid_col = T([128, 1], f32, name="grid_col")      # edges 0..G-1
            gridp_col = T([128, 1], f32, name="gridp_col")    # + (b1+b2)
            gridp_bf = T([128, 1], bf16, name="gridp_bf")
            riota = T([128, GE], i32, name="riota")
            riota_f = T([128, GE], f32, name="riota_f")
            grid_bc = T([128, GE], f32, name="grid_bc")
            bias_row = T([1, D], f32, name="bias_row")
            bias_bc = T([128, D], f32, name="bias_bc")

            xstage = T([128, B * IT * F], f32, name="xstage")  # raw X rows
            xbst = T([128, B * IT * F], bf16, name="xbst")     # bf16 cast
            xt = T([128, B, 2, NL], bf16, name="xt")          # X^T [f, b, fc, n]
            st_sb = T([128, B * NL], bf16, name="st_sb")      # S^T [d, (b,n)]
            tf_sb = T([2, B * NL], bf16, name="tf_sb")        # rows: (tau, f2)
            taui_bc = T([128, B * NL], bf16, name="taui_bc")  # tau bcast over parts
            a_bc = T([128, B * NL], bf16, name="a_bc")
            c_bc = T([128, B * NL], bf16, name="c_bc")
            sra = T([128, B * NL], bf16, name="sra")          # Sr * a  (lhsT)  [0:G used]
            src = T([128, B * NL], bf16, name="src")          # Sr * c  (lhsT)
            sc_st = T([128, 2 * B * IT], f32, name="sc_st")   # (tau, f2) cols
            bcol_st = T([128, B * IT], f32, name="bcol_st")
            dcol_st = T([128, B * IT], f32, name="dcol_st")
            ccol_st = T([128, B * IT], f32, name="ccol_st")
            m_bf = [T([G, WB], bf16, name=f"m_bf{b}") for b in range(B)]
            mrall = T([G, B * CORES * WB], bf16, name="mrall")
            mrb = [T([G, WB], bf16, name=f"mrb{b}") for b in range(B)]
            o_all = [T([128, IT * D], f32, name=f"o_all{b}") for b in range(B)]
            # bias-folded gather tables: S-cols + bias*den-col appended
            mbb = [T([G, D + 1], bf16, name=f"mbb{b}") for b in range(B)]
            mdb = [T([G, D + 1], bf16, name=f"mdb{b}") for b in range(B)]

            # ---------- small loads + constants ----------
            for fc in range(2):
                nc.sync.dma_start(
                    out=wf_sb[:, fc * D:(fc + 1) * D],
                    in_=wf_ext[fc * 128:(fc + 1) * 128, :],
                )
            nc.sync.dma_start(out=w12_st[:, 0:1], in_=w1_ext[:, :])
            nc.sync.dma_start(out=w12_st[:, 1:2], in_=w2_ext[:, :])
            nc.sync.dma_start(out=b1_sb, in_=b1_ext[:].unsqueeze(0))
            nc.sync.dma_start(out=b2_sb, in_=b2_ext[:].unsqueeze(0))
            nc.sync.dma_start(out=bias_row[:, :], in_=bias_ext[:].unsqueeze(0))

            # prefetch the full seq slab (one DMA per batch)
            for b in range(B):
                nc.sync.dma_start(
                    out=xstage[:, b * IT * F:(b + 1) * IT * F].rearrange(
                        "p (nt f) -> p nt f", nt=IT, f=F
                    ),
                    in_=seq_ext[b, :, :].rearrange("(nt p) f -> p nt f", p=128),
                )

            make_identity(nc, ident[:, :])
            nc.vector.memset(ones_row[:, :], 1.0)
            nc.vector.memset(ones_row_f[:, :], 1.0)
            nc.vector.memset(alpha_col[:, :], ALPHA)

            nc.vector.tensor_copy(wf_bf[:, :], wf_sb[:, :])
            nc.vector.tensor_scalar(
                out=w12_bf[:, 0:1], in0=w12_st[:, 0:1],
                scalar1=-1.0, scalar2=None, op0=ALU.mult,
            )
            nc.vector.tensor_copy(w12_bf[:, 1:2], w12_st[:, 1:2])

            nc.vector.tensor_tensor(out=b12, in0=b1_sb, in1=b2_sb, op=ALU.add)
            nc.gpsimd.partition_broadcast(b1_bc, b1_sb)
            nc.gpsimd.partition_broadcast(b2_bc, b2_sb)
            nc.gpsimd.partition_broadcast(b12_bc, b12)
            nc.vector.tensor_scalar(
                out=p2b1_bc, in0=b1_bc, scalar1=ALPHA, scalar2=None, op0=ALU.mult
            )
            nc.vector.tensor_scalar(
                out=p2b2_bc, in0=b2_bc, scalar1=ALPHA, scalar2=None, op0=ALU.mult
            )

            # grid: per-partition column (edges 0..G-1) and row (edges 0..G)
            nc.gpsimd.iota(giota[:, :], [[1, 1]], channel_multiplier=1)
            nc.vector.tensor_copy(giota_f[:, :], giota[:, :])
            nc.vector.tensor_scalar(
                out=grid_col[:, :], in0=giota_f[:, :],
                scalar1=GH, scalar2=LO, op0=ALU.mult, op1=ALU.add,
            )
            nc.vector.tensor_tensor(
                out=gridp_col[:, :], in0=grid_col[:, :], in1=b12_bc, op=ALU.add
            )
            nc.vector.tensor_copy(gridp_bf[:, :], gridp_col[:, :])
            # grid broadcast rows: iota along free dim, same on every partition
            nc.gpsimd.iota(riota[:, :], [[1, GE]], channel_multiplier=0)
            nc.vector.tensor_copy(riota_f[:, :], riota[:, :])
            nc.vector.tensor_scalar(
                out=grid_bc[:, :], in0=riota_f[:, :],
                scalar1=GH, scalar2=LO, op0=ALU.mult, op1=ALU.add,
            )

            with (
                tc.tile_pool(name="sn_pool", bufs=3) as sn_pool,
                tc.tile_pool(name="wbd_pool", bufs=3) as wbd_pool,
                tc.tile_pool(name="hs_pool", bufs=3) as hs_pool,
                tc.tile_pool(name="hb_pool", bufs=3) as hb_pool,
                tc.tile_pool(name="o_pool", bufs=4) as o_pool,
                tc.tile_pool(name="ph_psum", bufs=1, space="PSUM") as php,
                tc.tile_pool(name="mm_psum", bufs=1, space="PSUM") as pmm,
            ):
                # broadcast bias via PE outer product (f32, tiny)
                pbb = php.tile([128, 512], f32, tag="p512", bufs=2, name="pbb")
                nc.tensor.matmul(pbb[:, 0:D], lhsT=ones_row_f[:, :], rhs=bias_row[:, :])
                nc.scalar.copy(out=bias_bc[:, :], in_=pbb[:, 0:D])

                # ---------- stage 1: cast + transpose X (both batches) ----------
                for s in range(B * IT * F // 512):
                    nc.vector.tensor_copy(
                        xbst[:, s * 512:(s + 1) * 512],
                        xstage[:, s * 512:(s + 1) * 512],
                    )
                for b in range(B):
                    for nt in range(IT):
                        base = (b * IT + nt) * F
                        for fc in range(2):
                            pt = php.tile([128, 128], bf16, tag="mm128", bufs=2, name="pt")
                            nc.tensor.transpose(
                                pt[:, :],
                                xbst[:, base + fc * 128: base + (fc + 1) * 128],
                                ident[:, :],
                            )
                            dst = xt[:, b, fc, nt * 128:(nt + 1) * 128]
                            if fc == 0:
                                nc.scalar.copy(out=dst, in_=pt[:, :])
                            else:
                                nc.vector.tensor_copy(dst, pt[:, :])

                # ---------- stage 2: S^T + row columns (both batches) ----------
                for b in range(B):
                    for h in range(2):
                        pst = php.tile([128, 512], f32, tag="p512", bufs=2, name="pst")
                        for fc in range(2):
                            nc.tensor.matmul(
                                pst[:, :],
                                lhsT=wf_bf[:, fc * D:(fc + 1) * D],
                                rhs=xt[:, b, fc, h * 512:(h + 1) * 512],
                                start=(fc == 0),
                                stop=(fc == 1),
                            )
                        nc.scalar.copy(
                            out=st_sb[:, b * NL + h * 512: b * NL + (h + 1) * 512],
                            in_=pst[:, :],
                        )
                    # per-row-chunk (tau, f2) columns via S^T chunk @ [-w1|w2]
                    for nt in range(IT):
                        idx = b * IT + nt
                        psc = php.tile([128, 512], f32, tag="p512", bufs=2, name="psc")
                        nc.tensor.matmul(
                            psc[:, 0:2],
                            lhsT=st_sb[:, b * NL + nt * 128: b * NL + (nt + 1) * 128],
                            rhs=w12_bf[:, :],
                        )
                        nc.scalar.copy(
                            out=sc_st[:, 2 * idx: 2 * idx + 2], in_=psc[:, 0:2]
                        )
                    # column exps for this batch (strided slices)
                    cs = slice(b * IT, (b + 1) * IT)
                    f2cols = sc_st[:, 2 * b * IT + 1: 2 * (b + 1) * IT: 2]
                    taucols = sc_st[:, 2 * b * IT: 2 * (b + 1) * IT: 2]
                    nc.scalar.activation(
                        bcol_st[:, cs], f2cols, AF.Exp, scale=1.0, bias=b2_bc
                    )
                    nc.scalar.activation(
                        dcol_st[:, cs], f2cols, AF.Exp, scale=ALPHA, bias=p2b2_bc
                    )
                    nc.scalar.activation(
                        ccol_st[:, cs], taucols, AF.Exp, scale=-ALPHA, bias=p2b1_bc
                    )

                # ---------- stage 5: bucket tables, then one AllReduce ----------
                for b in range(B):
                    mps = pmm.tile([G, WB], f32, tag=f"mps{b}", bufs=1, name=f"mps{b}")
                    for nt in range(IT):
                        idx = b * IT + nt
                        # S natural chunk from S^T via PE transpose
                        pn = php.tile([128, 128], bf16, tag="mm128", bufs=2, name="pn")
                        nc.tensor.transpose(
                            pn[:, :],
                            st_sb[:, b * NL + nt * 128: b * NL + (nt + 1) * 128],
                            ident[:, :],
                        )
                        sn = sn_pool.tile([128, 128], bf16, tag="sn")
                        nc.vector.tensor_copy(sn[:, :], pn[:, :])
                        wbd = wbd_pool.tile([128, WB], bf16, tag="wbd")
                        nc.vector.tensor_scalar(
                            out=wbd[:, 0:D], in0=sn[:, :],
                            scalar1=bcol_st[:, idx:idx + 1], scalar2=None,
                            op0=ALU.mult,
                        )
                        nc.vector.tensor_copy(
                            wbd[:, D:D + 1], bcol_st[:, idx:idx + 1]
                        )
                        nc.vector.tensor_scalar(
                            out=wbd[:, D + 1:2 * D + 1], in0=sn[:, :],
                            scalar1=dcol_st[:, idx:idx + 1], scalar2=None,
                            op0=ALU.mult,
                        )
                        nc.vector.tensor_copy(
                            wbd[:, 2 * D + 1:WB], dcol_st[:, idx:idx + 1]
                        )
                        hs = hs_pool.tile([128, GE], bf16, tag="hs")
                        nc.vector.tensor_scalar(
                            out=hs[:, :], in0=grid_bc[:, :],
                            scalar1=sc_st[:, 2 * idx + 1: 2 * idx + 2], scalar2=None,
                            op0=ALU.is_le,
                        )
                        hb = hb_pool.tile([128, G], bf16, tag="hb")
                        nc.vector.tensor_tensor(
                            out=hb[:, :], in0=hs[:, 0:G], in1=hs[:, 1:GE],
                            op=ALU.subtract,
                        )
                        nc.tensor.matmul(
                            mps[:, :], lhsT=hb[:, :], rhs=wbd[:, :],
                            start=(nt == 0), stop=(nt == IT - 1),
                        )
                    nc.scalar.copy(out=m_bf[b][:, :], in_=mps[:, :])
                    nc.sync.dma_start(
                        out=cc_in[b * CCB:(b + 1) * CCB].rearrange(
                            "(p w) -> p w", p=G, w=WB
                        ),
                        in_=m_bf[b][:, :],
                    )

                # ---------- one AllGather for both batches ----------
                if skip_collective:
                    for r in range(CORES):
                        nc.sync.dma_start(
                            out=cc_out[r * B * CCB:(r + 1) * B * CCB],
                            in_=cc_in[:],
                        )
                else:
                    nc.gpsimd.collective_compute(
                        "AllGather",
                        ALU.bypass,
                        replica_groups=[list(range(CORES))],
                        ins=[cc_in[:].opt()],
                        outs=[cc_out[:].opt()],
                    )

                # ---------- stage 3 (overlaps AR): rows tau, broadcasts, Sr ----------
                for b in range(B):
                    bs = slice(b * NL, (b + 1) * NL)
                    # tau/f2 rows: [2, NL] = [-w1|w2]^T @ S^T  (512-col halves)
                    for q in range(2):
                        hs512 = slice(b * NL + q * 512, b * NL + (q + 1) * 512)
                        ptf = php.tile([128, 512], f32, tag="p512", bufs=2, name="ptf")
                        nc.tensor.matmul(
                            ptf[0:2, :], lhsT=w12_bf[:, :], rhs=st_sb[:, hs512]
                        )
                        nc.vector.tensor_copy(tf_sb[:, hs512], ptf[0:2, :])
                    # tau broadcast over partitions (PE outer), then a/c = exp
                    for q in range(2):
                        hs512 = slice(b * NL + q * 512, b * NL + (q + 1) * 512)
                        pbig = php.tile([128, 512], f32, tag="p512", bufs=2, name="pbig")
                        nc.tensor.matmul(
                            pbig[:, :], lhsT=ones_row[:, :], rhs=tf_sb[0:1, hs512]
                        )
                        nc.scalar.copy(out=taui_bc[:, hs512], in_=pbig[:, :])
                    nc.scalar.activation(
                        a_bc[:, bs], taui_bc[:, bs], AF.Exp, scale=-1.0, bias=b1_bc
                    )
                    nc.scalar.activation(
                        c_bc[:, bs], taui_bc[:, bs], AF.Exp, scale=-ALPHA, bias=p2b1_bc
                    )
                    # gather lhsT operands: sra = a*Sr (suffix), src = c*(1-Sr)
                    # (complement via is_gt: src @ Md directly yields the full
                    # C-branch sum — no FullD correction needed)
                    for q in range(2):
                        sl = slice(b * NL + q * 512, b * NL + (q + 1) * 512)
                        nc.vector.scalar_tensor_tensor(
                            out=sra[:, sl], in0=taui_bc[:, sl],
                            scalar=gridp_bf[:, 0:1], in1=a_bc[:, sl],
                            op0=ALU.is_le, op1=ALU.mult,
                        )
                        nc.vector.scalar_tensor_tensor(
                            out=src[:, sl], in0=taui_bc[:, sl],
                            scalar=gridp_bf[:, 0:1], in1=c_bc[:, sl],
                            op0=ALU.is_gt, op1=ALU.mult,
                        )

                # ---------- stage 6: gather partial tables + local sum ----------
                for b in range(B):
                    nc.sync.dma_start(
                        out=mrall[:, b * CORES * WB:(b + 1) * CORES * WB].rearrange(
                            "p (r w) -> p r w", r=CORES, w=WB
                        ),
                        in_=cc_out[:].rearrange(
                            "(r bb p w) -> bb p r w", r=CORES, bb=B, p=G, w=WB
                        )[b],
                    )
                mrv = mrall[:, :].rearrange(
                    "p (bb r w) -> p bb r w", bb=B, r=CORES, w=WB
                )
                for b in range(B):
                    nc.vector.tensor_tensor(
                        out=mrb[b][:, :], in0=mrv[:, b, 0, :],
                        in1=mrv[:, b, 1, :], op=ALU.add,
                    )
                    for r in range(2, CORES):
                        nc.vector.tensor_tensor(
                            out=mrb[b][:, :], in0=mrb[b][:, :],
                            in1=mrv[:, b, r, :], op=ALU.add,
                        )
                for b in range(B):
                    # fold bias into the tables: M'[g,d] = M[g,d] + den[g]*bias_d
                    # so po*zr directly yields vals+bias (division distributes)
                    nc.vector.scalar_tensor_tensor(
                        out=mbb[b][:, 0:D], in0=bias_bc[0:G, :],
                        scalar=mrb[b][:, D:D + 1], in1=mrb[b][:, 0:D],
                        op0=ALU.mult, op1=ALU.add,
                    )
                    nc.vector.tensor_copy(mbb[b][:, D:D + 1], mrb[b][:, D:D + 1])
                    nc.vector.scalar_tensor_tensor(
                        out=mdb[b][:, 0:D], in0=bias_bc[0:G, :],
                        scalar=mrb[b][:, WB - 1:WB], in1=mrb[b][:, D + 1:2 * D + 1],
                        op0=ALU.mult, op1=ALU.add,
                    )
                    nc.vector.tensor_copy(
                        mdb[b][:, D:D + 1], mrb[b][:, WB - 1:WB]
                    )
                for b in range(B):
                    for nt in range(IT):
                        ts = slice(b * NL + nt * 128, b * NL + (nt + 1) * 128)
                        po = pmm.tile(
                            [128, D + 1], f32, tag="po", bufs=2, name="po"
                        )
                        nc.tensor.matmul(
                            po[:, :], lhsT=sra[0:G, ts], rhs=mbb[b][:, :],
                            start=True, stop=False,
                        )
                        nc.tensor.matmul(
                            po[:, :], lhsT=src[0:G, ts], rhs=mdb[b][:, :],
                            start=False, stop=True,
                        )
                        zr = o_pool.tile([128, 1], f32, tag="zr")
                        nc.vector.reciprocal(zr[:, :], po[:, D:D + 1])
                        y = o_pool.tile([128, D], f32, tag="y")
                        nc.scalar.mul(y[:, :], po[:, 0:D], zr[:, 0:1])
                        nc.vector.scalar_tensor_tensor(
                            out=o_all[b][:, nt * D:(nt + 1) * D], in0=y[:, :],
                            scalar=alpha_col[:, 0:1],
                            in1=y[:, :], op0=ALU.mult, op1=ALU.max,
                        )
                    nc.sync.dma_start(
                        out=out_ext[b, :, :].rearrange(
                            "(nt p) d -> p nt d", p=128
                        ),
                        in_=o_all[b][:, :].rearrange(
                            "p (nt d) -> p nt d", d=D
                        ),
                    )

        persist_pool.__exit__(None, None, None)

    nc.compile()
    return nc


def _get_nc():
    if "nc" not in _cache:
        _cache["nc"] = build(
            skip_collective=bool(int(os.environ.get("SKIP_COLLECTIVE", "0")))
        )
    return _cache["nc"]


def kernel(seq, Wf, w1, b1, w2, b2, bias):
    from concourse.bass_utils import run_bass_kernel_spmd

    seq = np.ascontiguousarray(np.asarray(seq, dtype=np.float32))
    Wf = np.ascontiguousarray(np.asarray(Wf, dtype=np.float32))
    w1 = np.ascontiguousarray(np.asarray(w1, dtype=np.float32))
    b1 = np.ascontiguousarray(np.asarray(b1, dtype=np.float32))
    w2 = np.ascontiguousarray(np.asarray(w2, dtype=np.float32))
    b2 = np.ascontiguousarray(np.asarray(b2, dtype=np.float32))
    bias = np.ascontiguousarray(np.asarray(bias, dtype=np.float32))

    nc = _get_nc()
    in_maps = []
    for r in range(CORES):
        in_maps.append({
            "seq": np.ascontiguousarray(seq[:, r * NL:(r + 1) * NL, :]),
            "Wf": Wf, "w1": w1, "b1": b1, "w2": w2, "b2": b2, "bias": bias,
        })

    trace = bool(int(os.environ.get("KERNEL_TRACE", "0")))
    if trace:
        import concourse.bass_utils as bu
        bu.upload_artifacts = lambda tmpdir: ""  # no network in container

    res = run_bass_kernel_spmd(
        nc, in_maps, core_ids=list(range(CORES)), trace=trace
    )
    _cache["last_result"] = res
    _cache["exec_time_ns"] = res.exec_time_ns

    out = np.concatenate(
        [res.results[r]["out"] for r in range(CORES)], axis=1
    )
    return np.ascontiguousarray(out.astype(np.float32))
